# revision 1
# baseline (speedup 1.0000x reference)
"""Trainium2 Bass kernel for nn_LIFcomplexLayer.

Computes: Wx = x @ W.T ; BatchNorm(train stats over (B,T)) ; complex-decay
LIF recurrence with spike output.

Sharding: data-parallel over batch B=32 across 8 cores (4 each). BN statistics
are all-reduced across cores with a tiny [128, 2*HC] collective. Everything
runs in a single SPMD launch per core:
  phase A: stream x, PE-transpose 128x128 blocks, f32 matmuls -> Wx^T resident
           in SBUF laid out [h(128p), hc, b, t]; per-tile sums/sumsq partials.
  phase B: AllReduce stats, finalize BN scale/bias folded with b (drive
           coefficients), apply in-place on the Wx buffer via ACT.
  phase C: serial recurrence over T; spikes written in-place over the consumed
           drive column; DMA the spike buffer out.
"""

import sys

if "/opt/trn_rl_repo" not in sys.path:
    sys.path.insert(0, "/opt/trn_rl_repo")

import os
import numpy as np

B, T, I, H = 32, 2048, 512, 512
NCORES = 8
BLOC = B // NCORES          # 4 batches per core
P = 128                     # partitions
HC = H // P                 # 4 h-chunks
IC = I // P                 # 4 i-chunks
TC = 4                      # t-chunks per batch in phase A
TCH = T // TC               # 512 t per chunk
NTOT = float(B * T)         # BN sample count

TSTEPS = int(os.environ.get("LIF_TSTEPS", str(T)))

_CACHE = {}


def _build():
    import concourse.bass as bass
    import concourse.bacc as bacc
    import concourse.tile as tile
    from concourse import mybir

    dt = mybir.dt
    f32 = dt.float32
    Alu = mybir.AluOpType
    Act = mybir.ActivationFunctionType

    from contextlib import ExitStack

    nc = bacc.Bacc(
        "TRN2", target_bir_lowering=False, debug=False, num_devices=NCORES
    )

    x_d = nc.dram_tensor("x", [BLOC, T, I], f32, kind="ExternalInput").ap()
    wt_d = nc.dram_tensor("wt", [I, H], f32, kind="ExternalInput").ap()
    ident_d = nc.dram_tensor("ident", [P, P], f32, kind="ExternalInput").ap()
    arep_d = nc.dram_tensor("arep", [P, 3, HC, BLOC], f32, kind="ExternalInput").ap()
    bgh_d = nc.dram_tensor("bgh", [P, 2, HC], f32, kind="ExternalInput").ap()
    st0_d = nc.dram_tensor("st0", [P, 3, HC, BLOC], f32, kind="ExternalInput").ap()
    out_d = nc.dram_tensor("out", [HC, P, BLOC, T], f32, kind="ExternalOutput").ap()

    with tile.TileContext(nc) as tc, ExitStack() as ctx:
        consts = ctx.enter_context(tc.tile_pool(name="consts", bufs=1))
        big = ctx.enter_context(tc.tile_pool(name="big", bufs=1))
        xin = ctx.enter_context(tc.tile_pool(name="xin", bufs=3))
        xtp = ctx.enter_context(tc.tile_pool(name="xtp", bufs=2))
        ppool = ctx.enter_context(tc.tile_pool(name="psumT", bufs=4, space="PSUM"))
        mpool = ctx.enter_context(tc.tile_pool(name="psumM", bufs=2, space="PSUM"))
        trash_p = ctx.enter_context(tc.tile_pool(name="trash", bufs=2))
        small = ctx.enter_context(tc.tile_pool(name="small", bufs=1))
        state_p = ctx.enter_context(tc.tile_pool(name="state", bufs=4))
        scr = ctx.enter_context(tc.tile_pool(name="scr", bufs=3))
        dram = ctx.enter_context(tc.tile_pool(name="dram", bufs=1, space="DRAM"))

        wt_sb = consts.tile([P, IC, H], f32)
        nc.sync.dma_start(wt_sb[:], wt_d.rearrange("(ic p) h -> p ic h", p=P))
        ident_sb = consts.tile([P, P], f32)
        nc.sync.dma_start(ident_sb[:], ident_d[:])
        arep_sb = consts.tile([P, 3, HC, BLOC], f32)
        nc.sync.dma_start(arep_sb[:], arep_d[:])
        bgh_sb = consts.tile([P, 2, HC], f32)
        nc.sync.dma_start(bgh_sb[:], bgh_d[:])
        st0_sb = consts.tile([P, 3, HC, BLOC], f32)
        nc.sync.dma_start(st0_sb[:], st0_d[:])

        # Wx^T buffer, free dims (hc, b, t). Drive overwritten by spikes in C.
        wxbuf = big.tile([P, HC, BLOC, T], f32)
        sumS = small.tile([P, HC, BLOC * TC], f32)
        sumQ = small.tile([P, HC, BLOC * TC], f32)

        # ---- phase A ----
        for b in range(BLOC):
            for tcix in range(TC):
                xr = xin.tile([P, TC, I], f32)  # [t(128p), tt, i]
                nc.sync.dma_start(
                    xr[:],
                    x_d[b, tcix * TCH : (tcix + 1) * TCH, :].rearrange(
                        "(tt p) i -> p tt i", p=P
                    ),
                )
                xt = xtp.tile([P, IC, TCH], f32)  # [i(128p), ic, t]
                for tt in range(TC):
                    for ic in range(IC):
                        pt = ppool.tile([P, P], f32)
                        nc.tensor.transpose(
                            pt[:], xr[:, tt, ic * P : (ic + 1) * P], ident_sb[:]
                        )
                        nc.scalar.copy(xt[:, ic, tt * P : (tt + 1) * P], pt[:])
                idx = b * TC + tcix
                for hc in range(HC):
                    pm = mpool.tile([P, TCH], f32)
                    for ic in range(IC):
                        nc.tensor.matmul(
                            pm[:],
                            lhsT=wt_sb[:, ic, hc * P : (hc + 1) * P],
                            rhs=xt[:, ic, :],
                            start=(ic == 0),
                            stop=(ic == IC - 1),
                        )
                    dst = wxbuf[:, hc, b, tcix * TCH : (tcix + 1) * TCH]
                    nc.scalar.activation(
                        dst, pm[:], Act.Identity, accum_out=sumS[:, hc, idx : idx + 1]
                    )
                    trash = trash_p.tile([P, TCH], f32)
                    nc.vector.scalar_tensor_tensor(
                        trash[:],
                        dst,
                        1.0,
                        dst,
                        op0=Alu.bypass,
                        op1=Alu.mult,
                        accum_out=sumQ[:, hc, idx : idx + 1],
                    )

        # ---- phase B: stats all-reduce + BN finalize + drive prep ----
        stats = small.tile([P, 2, HC], f32)
        nc.vector.tensor_reduce(
            stats[:, 0, :], sumS[:], axis=mybir.AxisListType.X, op=Alu.add
        )
        nc.vector.tensor_reduce(
            stats[:, 1, :], sumQ[:], axis=mybir.AxisListType.X, op=Alu.add
        )
        cc_in = dram.tile([P, 2 * HC], f32)
        cc_out = dram.tile([P, 2 * HC], f32)
        nc.sync.dma_start(cc_in[:], stats[:].rearrange("p a h -> p (a h)"))
        nc.gpsimd.collective_compute(
            "AllReduce",
            Alu.add,
            replica_groups=[list(range(NCORES))],
            ins=[cc_in.opt()],
            outs=[cc_out.opt()],
        )
        gstats = small.tile([P, 2, HC], f32)
        nc.sync.dma_start(gstats[:], cc_out[:].rearrange("p (a h) -> p a h", a=2))

        mean = small.tile([P, HC], f32)
        ex2 = small.tile([P, HC], f32)
        var = small.tile([P, HC], f32)
        inv = small.tile([P, HC], f32)
        gsc = small.tile([P, HC], f32)
        hof = small.tile([P, HC], f32)
        tmp = small.tile([P, HC], f32)
        nc.vector.tensor_scalar(mean[:], gstats[:, 0, :], 1.0 / NTOT, None, op0=Alu.mult)
        nc.vector.tensor_scalar(ex2[:], gstats[:, 1, :], 1.0 / NTOT, None, op0=Alu.mult)
        nc.vector.tensor_tensor(tmp[:], mean[:], mean[:], op=Alu.mult)
        nc.vector.tensor_tensor(var[:], ex2[:], tmp[:], op=Alu.subtract)
        nc.vector.tensor_scalar(var[:], var[:], 1e-5, None, op0=Alu.add)
        nc.scalar.sqrt(tmp[:], var[:])
        nc.vector.reciprocal(inv[:], tmp[:])
        nc.vector.tensor_tensor(gsc[:], bgh_sb[:, 0, :], inv[:], op=Alu.mult)
        nc.vector.tensor_tensor(tmp[:], mean[:], gsc[:], op=Alu.mult)
        nc.vector.tensor_tensor(hof[:], bgh_sb[:, 1, :], tmp[:], op=Alu.subtract)

        for hc in range(HC):
            for b in range(BLOC):
                dst = wxbuf[:, hc, b, :]
                k = hc * BLOC + b
                if k % 3 == 0:
                    nc.scalar.activation(
                        dst, dst, Act.Identity,
                        bias=hof[:, hc : hc + 1], scale=gsc[:, hc : hc + 1],
                    )
                else:
                    eng = nc.vector if k % 3 == 1 else nc.gpsimd
                    eng.tensor_scalar(
                        dst, dst, gsc[:, hc : hc + 1], hof[:, hc : hc + 1],
                        op0=Alu.mult, op1=Alu.add,
                    )

        # ---- phase C: serial recurrence ----
        # Layout trick: raw ur_t overwrites the consumed drive column in wxbuf;
        # the spike threshold is applied in bulk after the loop. The spike
        # needed *inside* the loop is fused into one STT:
        #   negm_t = (ur_{t-1} > 0.5) - ur_{t-1}  = s_{t-1} - ur_{t-1}
        # Recurrence (signs folded into consts nar=-a_r, nai=-a_i):
        #   ur_t = nar*negm + (nai*ui_{t-1} + d_t)      ui_t = nai*negm + a_r*ui_{t-1}
        # DVE: negm, r1, p1, ur (loop cycle negm->p1->ur is DVE-only, 3 ops).
        # Pool: q, t2(in-place col), r2, ui.
        # State is w = -a_i*ui (scaled imag part), which removes one multiply:
        #   ur_t = nar*negm + (w_{t-1} + d_t)        w_t = aisq*negm + a_r*w_{t-1}
        nar = arep_sb[:, 0]   # [P, HC, BLOC]  -a_r
        aisq = arep_sb[:, 1]  # a_i^2
        arr = arep_sb[:, 2]   # a_r
        wA = state_p.tile([P, HC, BLOC], f32, tag="wA")
        wB = state_p.tile([P, HC, BLOC], f32, tag="wB")
        for t in range(TSTEPS):
            pw = st0_sb[:, 1] if t == 0 else (wA if (t - 1) % 2 == 0 else wB)[:]
            w = (wA if t % 2 == 0 else wB)[:]
            col = wxbuf[:, :, :, t]
            negm = scr.tile([P, HC, BLOC], f32, tag="negm")
            p1 = scr.tile([P, HC, BLOC], f32, tag="p1")
            rw = scr.tile([P, HC, BLOC], f32, tag="rw")
            rr = scr.tile([P, HC, BLOC], f32, tag="rr")
            nc.gpsimd.tensor_tensor(col, pw, col, op=Alu.add)
            if t == 0:
                nc.vector.tensor_tensor(
                    negm[:], st0_sb[:, 2], st0_sb[:, 0], op=Alu.subtract
                )
            else:
                pcol = wxbuf[:, :, :, t - 1]
                nc.vector.scalar_tensor_tensor(
                    negm[:], pcol, 0.5, pcol, op0=Alu.is_gt, op1=Alu.subtract
                )
            nc.vector.tensor_tensor(p1[:], nar, negm[:], op=Alu.mult)
            nc.vector.tensor_tensor(col, p1[:], col, op=Alu.add)
            nc.gpsimd.tensor_tensor(rr[:], arr, pw, op=Alu.mult)
            nc.gpsimd.tensor_tensor(rw[:], aisq, negm[:], op=Alu.mult)
            nc.gpsimd.tensor_tensor(w, rw[:], rr[:], op=Alu.add)

        # bulk spike over the stored ur values, then stream out. All threshold
        # ops are issued before any DMA so the DMAs don't serialize them.
        for hc in range(HC):
            for b in range(BLOC):
                row = wxbuf[:, hc, b, :]
                eng = nc.vector if (hc * BLOC + b) % 2 == 0 else nc.gpsimd
                eng.tensor_scalar(row, row, 0.5, None, op0=Alu.is_gt)
        for hc in range(HC):
            nc.sync.dma_start(out_d[hc], wxbuf[:, hc, :, :])

    nc.compile()
    return nc


def _prep_host(W, log_log_alpha, log_dt, alpha_img, b, gamma, beta):
    lla = np.float32(np.exp(log_log_alpha.astype(np.float32)))
    dtv = np.exp(log_dt.astype(np.float32)).astype(np.float32)
    z = (-lla.astype(np.complex64) + 1j * alpha_img.astype(np.complex64)) * dtv
    alpha = np.exp(z.astype(np.complex64))
    a_r = alpha.real.astype(np.float32)
    a_i = alpha.imag.astype(np.float32)

    wt = np.ascontiguousarray(W.T.astype(np.float32))  # [I, H]
    ident = np.eye(P, dtype=np.float32)

    def tohc(v):  # [H] -> [P, HC]
        return np.ascontiguousarray(v.reshape(HC, P).T.astype(np.float32))

    arep = np.zeros((P, 3, HC, BLOC), np.float32)
    arep[:, 0] = tohc(-a_r)[:, :, None]
    arep[:, 1] = tohc(a_i * a_i)[:, :, None]
    arep[:, 2] = tohc(a_r)[:, :, None]

    bgh = np.zeros((P, 2, HC), np.float32)
    bgh[:, 0] = tohc((b * gamma).astype(np.float32))
    bgh[:, 1] = tohc((b * beta).astype(np.float32))
    return wt, ident, arep, bgh, -a_i


def kernel(x, W, log_log_alpha, log_dt, alpha_img, b, gamma, beta,
           u0_real, u0_imag, s0):
    from concourse.bass_utils import run_bass_kernel_spmd

    if "nc" not in _CACHE:
        _CACHE["nc"] = _build()
    nc = _CACHE["nc"]

    wt, ident, arep, bgh, nai = _prep_host(
        W, log_log_alpha, log_dt, alpha_img, b, gamma, beta
    )

    w0 = u0_imag.astype(np.float32) * nai[None, :]  # scaled imag state
    in_maps = []
    for c in range(NCORES):
        bs = slice(c * BLOC, (c + 1) * BLOC)
        st0 = np.zeros((P, 3, HC, BLOC), np.float32)
        # [P, HC, BLOC] views of per-core initial state
        st0[:, 0] = u0_real[bs].astype(np.float32).T.reshape(HC, P, BLOC).transpose(1, 0, 2)
        st0[:, 1] = w0[bs].T.reshape(HC, P, BLOC).transpose(1, 0, 2)
        st0[:, 2] = s0[bs].astype(np.float32).T.reshape(HC, P, BLOC).transpose(1, 0, 2)
        in_maps.append({
            "x": np.ascontiguousarray(x[bs].astype(np.float32)),
            "wt": wt,
            "ident": ident,
            "arep": arep,
            "bgh": bgh,
            "st0": st0,
        })

    res = run_bass_kernel_spmd(
        nc,
        in_maps,
        core_ids=list(range(NCORES)),
        trace=bool(int(os.environ.get("LIF_TRACE", "0"))),
    )
    _CACHE["last_res"] = res
    out = np.empty((B, T, H), np.float32)
    for c in range(NCORES):
        o = res.results[c]["out"]  # [HC, P, BLOC, T]
        out[c * BLOC : (c + 1) * BLOC] = o.transpose(2, 3, 0, 1).reshape(
            BLOC, T, H
        )
    return out



# revision 7
# speedup vs baseline: 1.1777x; 1.1777x over previous
"""Trainium2 Bass kernel for nn_LIFcomplexLayer (v2: h-on-partition reshard).

Computes: Wx = x @ W.T ; BatchNorm(train stats over (B,T)) ; complex-decay
LIF recurrence with spike output.

Sharding: 8 cores = 4 h-chunks x 2 b-halves. Core c owns h in
[128*(c//2), 128*(c//2)+128) and batches [16*(c%2), 16*(c%2)+16).
With h on partitions, all per-neuron constants are per-partition [P,1]
scalars, so each recurrence step is 4 fused STT ops on one engine (DVE):
    cw   = Q*wt + d_t            (Q = a_i^2, wt = -u_i/a_i scaled imag state)
    wt'  = A*wt + negm           (A = a_r)
    v    = -A*negm + cw          (written into the drive column, in place)
    negm'= (v > 0.5) - v
Spike extraction ((v>0.5) in bulk) runs on GpSimd in T-chunks so the output
DMA overlaps the tail of the recurrence.

BN statistics are all-reduced across cores with a tiny [128, 8] collective
(each core fills its h-chunk slot; the pair of cores sharing an h-chunk sum).
"""

import sys

if "/opt/trn_rl_repo" not in sys.path:
    sys.path.insert(0, "/opt/trn_rl_repo")

import os
import numpy as np

B, T, I, H = 32, 2048, 512, 512
NCORES = 8
NH = 4                       # h-shards
NB = 2                       # b-shards
BLOC = B // NB               # 16 batches per core
P = 128                      # partitions = h per core
IC = I // P                  # 4 i-chunks
TC = 4                       # t-chunks per batch in phase A
TCH = T // TC                # 512 t per chunk
NTOT = float(B * T)          # BN sample count
BN_EPS = 1e-5

TSTEPS = int(os.environ.get("LIF_TSTEPS", str(T)))
OCH = 512                    # spike-output chunk (steps per out DMA)

_CACHE = {}


def _build():
    import concourse.bass as bass
    import concourse.bacc as bacc
    import concourse.tile as tile
    from concourse import mybir

    dt = mybir.dt
    f32 = dt.float32
    Alu = mybir.AluOpType
    Act = mybir.ActivationFunctionType

    from contextlib import ExitStack

    nc = bacc.Bacc(
        "TRN2", target_bir_lowering=False, debug=False, num_devices=NCORES
    )

    x_d = nc.dram_tensor("x", [BLOC, T, I], f32, kind="ExternalInput").ap()
    wt_d = nc.dram_tensor("wt", [I, P], f32, kind="ExternalInput").ap()
    ident_d = nc.dram_tensor("ident", [P, P], f32, kind="ExternalInput").ap()
    cvec_d = nc.dram_tensor("cvec", [P, 8], f32, kind="ExternalInput").ap()
    st0_d = nc.dram_tensor("st0", [P, 2, BLOC], f32, kind="ExternalInput").ap()
    hsel_d = nc.dram_tensor("hsel", [P, NH], f32, kind="ExternalInput").ap()
    out_d = nc.dram_tensor("out", [P, BLOC, T], f32, kind="ExternalOutput").ap()

    with tile.TileContext(nc) as tc, ExitStack() as ctx:
        consts = ctx.enter_context(tc.tile_pool(name="consts", bufs=1))
        big = ctx.enter_context(tc.tile_pool(name="big", bufs=1))
        xin = ctx.enter_context(tc.tile_pool(name="xin", bufs=2))
        xtp = ctx.enter_context(tc.tile_pool(name="xtp", bufs=2))
        tpool = ctx.enter_context(tc.tile_pool(name="psumT", bufs=4, space="PSUM"))
        mpool = ctx.enter_context(tc.tile_pool(name="psumM", bufs=2, space="PSUM"))
        trash_p = ctx.enter_context(tc.tile_pool(name="trash", bufs=2))
        small = ctx.enter_context(tc.tile_pool(name="small", bufs=1))
        state_p = ctx.enter_context(tc.tile_pool(name="state", bufs=1))
        scr = ctx.enter_context(tc.tile_pool(name="scr", bufs=2))
        dram = ctx.enter_context(tc.tile_pool(name="dram", bufs=1, space="DRAM"))

        wt_sb = consts.tile([P, IC, P], f32)        # [i(128p), ic, h]
        nc.sync.dma_start(wt_sb[:], wt_d.rearrange("(ic p) h -> p ic h", p=P))
        ident_sb = consts.tile([P, P], f32)
        nc.sync.dma_start(ident_sb[:], ident_d[:])
        cvec = consts.tile([P, 8], f32)             # A, nA, Q, bg, bb, ...
        nc.sync.dma_start(cvec[:], cvec_d[:])
        st0_sb = consts.tile([P, 2, BLOC], f32)     # negm0, wtil0
        nc.sync.dma_start(st0_sb[:], st0_d[:])
        hsel = consts.tile([P, NH], f32)            # one-hot column = my hc
        nc.sync.dma_start(hsel[:], hsel_d[:])

        A_ap = cvec[:, 0:1]
        nA_ap = cvec[:, 1:2]
        Q_ap = cvec[:, 2:3]
        bg_ap = cvec[:, 3:4]   # b * gamma
        bb_ap = cvec[:, 4:5]   # b * beta

        # Drive buffer: [h(128p), b, t]. Raw Wx -> BN'd drive -> v -> spikes.
        wxbuf = big.tile([P, BLOC, T], f32)
        sumS = small.tile([P, BLOC * TC], f32)
        sumQ = small.tile([P, BLOC * TC], f32)

        # ---- phase A: x load, PE transpose, matmul, raw sums ----
        for b in range(BLOC):
            for tcix in range(TC):
                xr = xin.tile([P, TC, I], f32)  # [t(128p), tt, i]
                nc.sync.dma_start(
                    xr[:],
                    x_d[b, tcix * TCH : (tcix + 1) * TCH, :].rearrange(
                        "(tt p) i -> p tt i", p=P
                    ),
                )
                xt = xtp.tile([P, IC, TCH], f32)  # [i(128p), ic, t]
                k = b * TC + tcix
                for ic in range(IC):
                    pt = tpool.tile([P, TCH], f32)
                    for tt in range(TC):
                        nc.tensor.transpose(
                            pt[:, tt * P : (tt + 1) * P],
                            xr[:, tt, ic * P : (ic + 1) * P],
                            ident_sb[:],
                        )
                    # gpsimd cannot read PSUM; rotate ACT/DVE only
                    if (k * IC + ic) % 2 == 0:
                        nc.scalar.copy(xt[:, ic, :], pt[:])
                    else:
                        nc.vector.tensor_copy(xt[:, ic, :], pt[:])
                pm = mpool.tile([P, TCH], f32)
                for ic in range(IC):
                    nc.tensor.matmul(
                        pm[:],
                        lhsT=wt_sb[:, ic, :],
                        rhs=xt[:, ic, :],
                        start=(ic == 0),
                        stop=(ic == IC - 1),
                    )
                dst = wxbuf[:, b, tcix * TCH : (tcix + 1) * TCH]
                nc.scalar.activation(
                    dst, pm[:], Act.Identity, accum_out=sumS[:, k : k + 1]
                )
                trash = trash_p.tile([P, TCH], f32)
                nc.vector.scalar_tensor_tensor(
                    trash[:],
                    dst,
                    1.0,
                    dst,
                    op0=Alu.bypass,
                    op1=Alu.mult,
                    accum_out=sumQ[:, k : k + 1],
                )

        # ---- phase B: stats all-reduce + BN finalize + apply ----
        stats = small.tile([P, 2, NH], f32)
        nc.vector.memset(stats[:], 0.0)
        ssum = small.tile([P, 2], f32)
        nc.vector.tensor_reduce(
            ssum[:, 0:1], sumS[:], axis=mybir.AxisListType.X, op=Alu.add
        )
        nc.vector.tensor_reduce(
            ssum[:, 1:2], sumQ[:], axis=mybir.AxisListType.X, op=Alu.add
        )
        # scatter into my h-chunk slot: stats[:, s, hc] = ssum[:, s]
        nc.vector.tensor_scalar(
            stats[:, 0, :], hsel[:], ssum[:, 0:1], None, op0=Alu.mult
        )
        nc.vector.tensor_scalar(
            stats[:, 1, :], hsel[:], ssum[:, 1:2], None, op0=Alu.mult
        )
        cc_in = dram.tile([P, 2 * NH], f32)
        cc_out = dram.tile([P, 2 * NH], f32)
        nc.sync.dma_start(cc_in[:], stats[:].rearrange("p a h -> p (a h)"))
        nc.gpsimd.collective_compute(
            "AllReduce",
            Alu.add,
            replica_groups=[list(range(NCORES))],
            ins=[cc_in.opt()],
            outs=[cc_out.opt()],
        )
        gstats = small.tile([P, 2, NH], f32)
        nc.sync.dma_start(gstats[:], cc_out[:].rearrange("p (a h) -> p a h", a=2))
        gsum = small.tile([P, 2], f32)
        # gather my slot back: multiply by one-hot and reduce
        tmp2 = small.tile([P, NH], f32)
        nc.vector.tensor_tensor(tmp2[:], gstats[:, 0, :], hsel[:], op=Alu.mult)
        nc.vector.tensor_reduce(
            gsum[:, 0:1], tmp2[:], axis=mybir.AxisListType.X, op=Alu.add
        )
        nc.vector.tensor_tensor(tmp2[:], gstats[:, 1, :], hsel[:], op=Alu.mult)
        nc.vector.tensor_reduce(
            gsum[:, 1:2], tmp2[:], axis=mybir.AxisListType.X, op=Alu.add
        )

        mean = small.tile([P, 1], f32)
        ex2 = small.tile([P, 1], f32)
        var = small.tile([P, 1], f32)
        inv = small.tile([P, 1], f32)
        gsc = small.tile([P, 1], f32)
        hof = small.tile([P, 1], f32)
        tmp = small.tile([P, 1], f32)
        nc.vector.tensor_scalar(mean[:], gsum[:, 0:1], 1.0 / NTOT, None, op0=Alu.mult)
        nc.vector.tensor_scalar(ex2[:], gsum[:, 1:2], 1.0 / NTOT, None, op0=Alu.mult)
        nc.vector.tensor_tensor(tmp[:], mean[:], mean[:], op=Alu.mult)
        nc.vector.tensor_tensor(var[:], ex2[:], tmp[:], op=Alu.subtract)
        nc.vector.tensor_scalar(var[:], var[:], BN_EPS, None, op0=Alu.add)
        nc.scalar.sqrt(tmp[:], var[:])
        nc.vector.reciprocal(inv[:], tmp[:])
        nc.vector.tensor_tensor(gsc[:], bg_ap, inv[:], op=Alu.mult)
        nc.vector.tensor_tensor(tmp[:], mean[:], gsc[:], op=Alu.mult)
        nc.vector.tensor_tensor(hof[:], bb_ap, tmp[:], op=Alu.subtract)

        for b in range(BLOC):
            row = wxbuf[:, b, :]
            eng = (nc.scalar, nc.vector, nc.gpsimd)[b % 3]
            if eng is nc.scalar:
                nc.scalar.activation(
                    row, row, Act.Identity, bias=hof[:], scale=gsc[:]
                )
            else:
                eng.tensor_scalar(
                    row, row, gsc[:], hof[:], op0=Alu.mult, op1=Alu.add
                )

        # ---- phase C: serial recurrence ----
        # Step 0 uses the explicit (negm, wtil) form; steps >= 1 keep the
        # voltage state in the previous wxbuf column and run 3 DVE ops/step:
        #   cw    = Q*wtil + d_t                      (stock STT)
        #   wtil' = A*wtil + ((v_prev>0.5) - v_prev)  (custom LIF_W)
        #   v     = cw - A*((v_prev>0.5) - v_prev)    (custom LIF_V)
        from lif_ops import register_ops

        LIF_V, LIF_W = register_ops()

        negm = state_p.tile([P, BLOC], f32)
        wtil = state_p.tile([P, BLOC], f32)
        nc.vector.tensor_copy(negm[:], st0_sb[:, 0, :])
        nc.vector.tensor_copy(wtil[:], st0_sb[:, 1, :])

        # step 0 (explicit negm/wtil)
        col0 = wxbuf[:, :, 0]
        cw = scr.tile([P, BLOC], f32, tag="cw")
        nc.vector.scalar_tensor_tensor(
            cw[:], wtil[:], Q_ap, col0, op0=Alu.mult, op1=Alu.add
        )
        nc.vector.scalar_tensor_tensor(
            wtil[:], wtil[:], A_ap, negm[:], op0=Alu.mult, op1=Alu.add
        )
        nc.vector.scalar_tensor_tensor(
            col0, negm[:], nA_ap, cw[:], op0=Alu.mult, op1=Alu.add
        )

        nch = (TSTEPS + OCH - 1) // OCH
        for ch in range(nch):
            t0, t1 = ch * OCH, min((ch + 1) * OCH, TSTEPS)
            for t in range(t0, t1):
                if t == 0:
                    continue
                vprev = wxbuf[:, :, t - 1]
                col = wxbuf[:, :, t]
                cw = scr.tile([P, BLOC], f32, tag="cw")
                nc.vector.scalar_tensor_tensor(
                    cw[:], wtil[:], Q_ap, col, op0=Alu.mult, op1=Alu.add
                )
                nc.vector._custom_dve(
                    LIF_W, out=wtil[:], in0=vprev, in1=wtil[:], s0=A_ap, s1=0.5
                )
                nc.vector._custom_dve(
                    LIF_V, out=col, in0=vprev, in1=cw[:], s0=A_ap, s1=0.5
                )
            # bulk spike conversion for this chunk on GpSimd (overlaps DVE),
            # then stream the chunk out.
            for b in range(BLOC):
                seg = wxbuf[:, b, t0:t1]
                nc.gpsimd.tensor_scalar(seg, seg, 0.5, None, op0=Alu.is_gt)
            nc.sync.dma_start(out_d[:, :, t0:t1], wxbuf[:, :, t0:t1])

    nc.compile()
    return nc


def _prep_host(W, log_log_alpha, log_dt, alpha_img, b, gamma, beta):
    lla = np.float32(np.exp(log_log_alpha.astype(np.float32)))
    dtv = np.exp(log_dt.astype(np.float32)).astype(np.float32)
    z = (-lla.astype(np.complex64) + 1j * alpha_img.astype(np.complex64)) * dtv
    alpha = np.exp(z.astype(np.complex64))
    a_r = alpha.real.astype(np.float32)
    a_i = alpha.imag.astype(np.float32)
    ident = np.eye(P, dtype=np.float32)
    return a_r, a_i, ident


def kernel(x, W, log_log_alpha, log_dt, alpha_img, b, gamma, beta,
           u0_real, u0_imag, s0):
    from concourse.bass_utils import run_bass_kernel_spmd

    if "nc" not in _CACHE:
        _CACHE["nc"] = _build()
    nc = _CACHE["nc"]

    a_r, a_i, ident = _prep_host(W, log_log_alpha, log_dt, alpha_img, b, gamma, beta)

    in_maps = []
    for c in range(NCORES):
        hc, bh = c // NB, c % NB
        hs = slice(hc * P, (hc + 1) * P)
        bs = slice(bh * BLOC, (bh + 1) * BLOC)
        arh = a_r[hs]
        aih = a_i[hs]
        cvec = np.zeros((P, 8), np.float32)
        cvec[:, 0] = arh
        cvec[:, 1] = -arh
        cvec[:, 2] = aih * aih
        cvec[:, 3] = (b * gamma)[hs].astype(np.float32)
        cvec[:, 4] = (b * beta)[hs].astype(np.float32)
        st0 = np.zeros((P, 2, BLOC), np.float32)
        st0[:, 0] = (
            s0[bs].astype(np.float32) - u0_real[bs].astype(np.float32)
        ).T[hs, :]
        st0[:, 1] = (-u0_imag[bs].astype(np.float32)).T[hs, :] / aih[:, None]
        hsel = np.zeros((P, NH), np.float32)
        hsel[:, hc] = 1.0
        in_maps.append({
            "x": np.ascontiguousarray(x[bs].astype(np.float32)),
            "wt": np.ascontiguousarray(W[hs, :].T.astype(np.float32)),
            "ident": ident,
            "cvec": cvec,
            "st0": st0,
            "hsel": hsel,
        })

    res = run_bass_kernel_spmd(
        nc,
        in_maps,
        core_ids=list(range(NCORES)),
        trace=bool(int(os.environ.get("LIF_TRACE", "0"))),
    )
    _CACHE["last_res"] = res
    out = np.empty((B, T, H), np.float32)
    for c in range(NCORES):
        hc, bh = c // NB, c % NB
        o = res.results[c]["out"]  # [P, BLOC, T]
        out[bh * BLOC : (bh + 1) * BLOC, :, hc * P : (hc + 1) * P] = o.transpose(
            1, 2, 0
        )
    return out


# revision 13
# speedup vs baseline: 1.6370x; 1.3899x over previous
"""Trainium2 Bass kernel for nn_LIFcomplexLayer (v2: h-on-partition reshard).

Computes: Wx = x @ W.T ; BatchNorm(train stats over (B,T)) ; complex-decay
LIF recurrence with spike output.

Sharding: 8 cores = 4 h-chunks x 2 b-halves. Core c owns h in
[128*(c//2), 128*(c//2)+128) and batches [16*(c%2), 16*(c%2)+16).
With h on partitions, all per-neuron constants are per-partition [P,1]
scalars, so each recurrence step is 4 fused STT ops on one engine (DVE):
    cw   = Q*wt + d_t            (Q = a_i^2, wt = -u_i/a_i scaled imag state)
    wt'  = A*wt + negm           (A = a_r)
    v    = -A*negm + cw          (written into the drive column, in place)
    negm'= (v > 0.5) - v
Spike extraction ((v>0.5) in bulk) runs on GpSimd in T-chunks so the output
DMA overlaps the tail of the recurrence.

BN statistics are all-reduced across cores with a tiny [128, 8] collective
(each core fills its h-chunk slot; the pair of cores sharing an h-chunk sum).
"""

import sys

if "/opt/trn_rl_repo" not in sys.path:
    sys.path.insert(0, "/opt/trn_rl_repo")

import os
import numpy as np

B, T, I, H = 32, 2048, 512, 512
NCORES = 8
NH = 4                       # h-shards
NB = 2                       # b-shards
BLOC = B // NB               # 16 batches per core
P = 128                      # partitions = h per core
IC = I // P                  # 4 i-chunks
TC = 4                       # t-chunks per batch in phase A
TCH = T // TC                # 512 t per chunk
NTOT = float(B * T)          # BN sample count
BN_EPS = 1e-5

TSTEPS = int(os.environ.get("LIF_TSTEPS", str(T)))
OCH = 512                    # spike-output chunk (steps per out DMA)

_CACHE = {}


def _build():
    import concourse.bass as bass
    import concourse.bacc as bacc
    import concourse.tile as tile
    from concourse import mybir

    dt = mybir.dt
    f32 = dt.float32
    Alu = mybir.AluOpType
    Act = mybir.ActivationFunctionType

    from contextlib import ExitStack

    nc = bacc.Bacc(
        "TRN2", target_bir_lowering=False, debug=False, num_devices=NCORES
    )

    x_d = nc.dram_tensor("x", [BLOC, T, I], f32, kind="ExternalInput").ap()
    wt_d = nc.dram_tensor("wt", [I, P], f32, kind="ExternalInput").ap()
    ident_d = nc.dram_tensor("ident", [P, P], f32, kind="ExternalInput").ap()
    cvec_d = nc.dram_tensor("cvec", [P, 8], f32, kind="ExternalInput").ap()
    st0_d = nc.dram_tensor("st0", [P, 2, BLOC], f32, kind="ExternalInput").ap()
    hsel_d = nc.dram_tensor("hsel", [P, NH], f32, kind="ExternalInput").ap()
    out_d = nc.dram_tensor("out", [P, T, BLOC], f32, kind="ExternalOutput").ap()

    with tile.TileContext(nc) as tc, ExitStack() as ctx:
        consts = ctx.enter_context(tc.tile_pool(name="consts", bufs=1))
        big = ctx.enter_context(tc.tile_pool(name="big", bufs=1))
        xin = ctx.enter_context(tc.tile_pool(name="xin", bufs=2))
        xtp = ctx.enter_context(tc.tile_pool(name="xtp", bufs=2))
        tpool = ctx.enter_context(tc.tile_pool(name="psumT", bufs=4, space="PSUM"))
        mpool = ctx.enter_context(tc.tile_pool(name="psumM", bufs=2, space="PSUM"))
        trash_p = ctx.enter_context(tc.tile_pool(name="trash", bufs=2))
        small = ctx.enter_context(tc.tile_pool(name="small", bufs=1))
        state_p = ctx.enter_context(tc.tile_pool(name="state", bufs=1))
        scr = ctx.enter_context(tc.tile_pool(name="scr", bufs=2))
        dram = ctx.enter_context(tc.tile_pool(name="dram", bufs=1, space="DRAM"))

        wt_sb = consts.tile([P, IC, P], f32)        # [i(128p), ic, h]
        nc.sync.dma_start(wt_sb[:], wt_d.rearrange("(ic p) h -> p ic h", p=P))
        ident_sb = consts.tile([P, P], f32)
        nc.sync.dma_start(ident_sb[:], ident_d[:])
        cvec = consts.tile([P, 8], f32)             # A, nA, Q, bg, bb, ...
        nc.sync.dma_start(cvec[:], cvec_d[:])
        st0_sb = consts.tile([P, 2, BLOC], f32)     # negm0, wtil0
        nc.sync.dma_start(st0_sb[:], st0_d[:])
        hsel = consts.tile([P, NH], f32)            # one-hot column = my hc
        nc.sync.dma_start(hsel[:], hsel_d[:])

        A_ap = cvec[:, 0:1]
        nA_ap = cvec[:, 1:2]
        Q_ap = cvec[:, 2:3]
        bg_ap = cvec[:, 3:4]   # b * gamma
        bb_ap = cvec[:, 4:5]   # b * beta

        # Drive buffer: [h(128p), t, b] (t-major so per-step [P, BLOC] tiles
        # are contiguous). Raw Wx -> BN'd drive -> v -> spikes.
        wxbuf = big.tile([P, T, BLOC], f32)
        sumS = small.tile([P, BLOC * TC], f32)
        sumQ = small.tile([P, BLOC * TC], f32)

        # ---- phase A: x load, PE transpose, matmul, raw sums ----
        for b in range(BLOC):
            for tcix in range(TC):
                xr = xin.tile([P, TC, I], f32)  # [t(128p), tt, i]
                nc.sync.dma_start(
                    xr[:],
                    x_d[b, tcix * TCH : (tcix + 1) * TCH, :].rearrange(
                        "(tt p) i -> p tt i", p=P
                    ),
                )
                xt = xtp.tile([P, IC, TCH], f32)  # [i(128p), ic, t]
                k = b * TC + tcix
                for ic in range(IC):
                    pt = tpool.tile([P, TCH], f32)
                    for tt in range(TC):
                        nc.tensor.transpose(
                            pt[:, tt * P : (tt + 1) * P],
                            xr[:, tt, ic * P : (ic + 1) * P],
                            ident_sb[:],
                        )
                    # gpsimd cannot read PSUM; rotate ACT/DVE only
                    if (k * IC + ic) % 2 == 0:
                        nc.scalar.copy(xt[:, ic, :], pt[:])
                    else:
                        nc.vector.tensor_copy(xt[:, ic, :], pt[:])
                pm = mpool.tile([P, TCH], f32)
                for ic in range(IC):
                    nc.tensor.matmul(
                        pm[:],
                        lhsT=wt_sb[:, ic, :],
                        rhs=xt[:, ic, :],
                        start=(ic == 0),
                        stop=(ic == IC - 1),
                    )
                dst = wxbuf[:, tcix * TCH : (tcix + 1) * TCH, b]
                nc.scalar.activation(
                    dst, pm[:], Act.Identity, accum_out=sumS[:, k : k + 1]
                )
                trash = trash_p.tile([P, TCH], f32)
                nc.vector.scalar_tensor_tensor(
                    trash[:],
                    dst,
                    1.0,
                    dst,
                    op0=Alu.bypass,
                    op1=Alu.mult,
                    accum_out=sumQ[:, k : k + 1],
                )

        # ---- phase B: stats all-reduce + BN finalize + apply ----
        stats = small.tile([P, 2, NH], f32)
        nc.vector.memset(stats[:], 0.0)
        ssum = small.tile([P, 2], f32)
        nc.vector.tensor_reduce(
            ssum[:, 0:1], sumS[:], axis=mybir.AxisListType.X, op=Alu.add
        )
        nc.vector.tensor_reduce(
            ssum[:, 1:2], sumQ[:], axis=mybir.AxisListType.X, op=Alu.add
        )
        # scatter into my h-chunk slot: stats[:, s, hc] = ssum[:, s]
        nc.vector.tensor_scalar(
            stats[:, 0, :], hsel[:], ssum[:, 0:1], None, op0=Alu.mult
        )
        nc.vector.tensor_scalar(
            stats[:, 1, :], hsel[:], ssum[:, 1:2], None, op0=Alu.mult
        )
        cc_in = dram.tile([P, 2 * NH], f32)
        cc_out = dram.tile([P, 2 * NH], f32)
        nc.sync.dma_start(cc_in[:], stats[:].rearrange("p a h -> p (a h)"))
        nc.gpsimd.collective_compute(
            "AllReduce",
            Alu.add,
            replica_groups=[list(range(NCORES))],
            ins=[cc_in.opt()],
            outs=[cc_out.opt()],
        )
        gstats = small.tile([P, 2, NH], f32)
        nc.sync.dma_start(gstats[:], cc_out[:].rearrange("p (a h) -> p a h", a=2))
        gsum = small.tile([P, 2], f32)
        # gather my slot back: multiply by one-hot and reduce
        tmp2 = small.tile([P, NH], f32)
        nc.vector.tensor_tensor(tmp2[:], gstats[:, 0, :], hsel[:], op=Alu.mult)
        nc.vector.tensor_reduce(
            gsum[:, 0:1], tmp2[:], axis=mybir.AxisListType.X, op=Alu.add
        )
        nc.vector.tensor_tensor(tmp2[:], gstats[:, 1, :], hsel[:], op=Alu.mult)
        nc.vector.tensor_reduce(
            gsum[:, 1:2], tmp2[:], axis=mybir.AxisListType.X, op=Alu.add
        )

        mean = small.tile([P, 1], f32)
        ex2 = small.tile([P, 1], f32)
        var = small.tile([P, 1], f32)
        inv = small.tile([P, 1], f32)
        gsc = small.tile([P, 1], f32)
        hof = small.tile([P, 1], f32)
        tmp = small.tile([P, 1], f32)
        nc.vector.tensor_scalar(mean[:], gsum[:, 0:1], 1.0 / NTOT, None, op0=Alu.mult)
        nc.vector.tensor_scalar(ex2[:], gsum[:, 1:2], 1.0 / NTOT, None, op0=Alu.mult)
        nc.vector.tensor_tensor(tmp[:], mean[:], mean[:], op=Alu.mult)
        nc.vector.tensor_tensor(var[:], ex2[:], tmp[:], op=Alu.subtract)
        nc.vector.tensor_scalar(var[:], var[:], BN_EPS, None, op0=Alu.add)
        nc.scalar.sqrt(tmp[:], var[:])
        nc.vector.reciprocal(inv[:], tmp[:])
        nc.vector.tensor_tensor(gsc[:], bg_ap, inv[:], op=Alu.mult)
        nc.vector.tensor_tensor(tmp[:], mean[:], gsc[:], op=Alu.mult)
        nc.vector.tensor_tensor(hof[:], bb_ap, tmp[:], op=Alu.subtract)

        # BN apply on big contiguous [P, TCH*BLOC] slices, DVE/ACT alternating
        for tcix in range(TC):
            sl = wxbuf[:, tcix * TCH : (tcix + 1) * TCH, :]
            if tcix % 2 == 0:
                nc.vector.tensor_scalar(
                    sl, sl, gsc[:], hof[:], op0=Alu.mult, op1=Alu.add
                )
            else:
                nc.scalar.activation(
                    sl, sl, Act.Identity, bias=hof[:], scale=gsc[:]
                )

        # ---- phase C: serial recurrence ----
        # Step 0 uses the explicit (negm, wtil) form; steps >= 1 keep the
        # voltage state in the previous wxbuf column and run 3 DVE ops/step:
        #   cw    = Q*wtil + d_t                      (stock STT)
        #   wtil' = A*wtil + ((v_prev>0.5) - v_prev)  (custom LIF_W)
        #   v     = cw - A*((v_prev>0.5) - v_prev)    (custom LIF_V)
        from lif_ops import register_ops

        LIF_V, LIF_W = register_ops()

        negm = state_p.tile([P, BLOC], f32)
        wtil = state_p.tile([P, BLOC], f32)
        nc.vector.tensor_copy(negm[:], st0_sb[:, 0, :])
        nc.vector.tensor_copy(wtil[:], st0_sb[:, 1, :])

        # step 0 (explicit negm/wtil)
        col0 = wxbuf[:, 0, :]
        cw = scr.tile([P, BLOC], f32, tag="cw")
        nc.vector.scalar_tensor_tensor(
            cw[:], wtil[:], Q_ap, col0, op0=Alu.mult, op1=Alu.add
        )
        nc.vector.scalar_tensor_tensor(
            wtil[:], wtil[:], A_ap, negm[:], op0=Alu.mult, op1=Alu.add
        )
        nc.vector.scalar_tensor_tensor(
            col0, negm[:], nA_ap, cw[:], op0=Alu.mult, op1=Alu.add
        )

        # Spike conversion lags one column behind the recurrence (the next
        # step reads raw v from the previous column), so chunk ch converts
        # and DMAs [t0-1, t1-1); the final column is flushed after the loop.
        nch = (TSTEPS + OCH - 1) // OCH
        for ch in range(nch):
            t0, t1 = ch * OCH, min((ch + 1) * OCH, TSTEPS)
            for t in range(t0, t1):
                if t == 0:
                    continue
                vprev = wxbuf[:, t - 1, :]
                col = wxbuf[:, t, :]
                cw = scr.tile([P, BLOC], f32, tag="cw")
                nc.vector.scalar_tensor_tensor(
                    cw[:], wtil[:], Q_ap, col, op0=Alu.mult, op1=Alu.add
                )
                nc.vector._custom_dve(
                    LIF_W, out=wtil[:], in0=vprev, in1=wtil[:], s0=A_ap, s1=0.5
                )
                nc.vector._custom_dve(
                    LIF_V, out=col, in0=vprev, in1=cw[:], s0=A_ap, s1=0.5
                )
            clo = max(t0 - 1, 0)
            chi = t1 - 1
            if chi > clo:
                seg = wxbuf[:, clo:chi, :]
                nc.vector.tensor_scalar(seg, seg, 0.5, None, op0=Alu.is_gt)
                nc.sync.dma_start(out_d[:, clo:chi, :], seg)
        # flush the last column
        seg = wxbuf[:, TSTEPS - 1 : TSTEPS, :]
        nc.vector.tensor_scalar(seg, seg, 0.5, None, op0=Alu.is_gt)
        nc.sync.dma_start(out_d[:, TSTEPS - 1 : TSTEPS, :], seg)

    nc.compile()
    return nc


def _prep_host(W, log_log_alpha, log_dt, alpha_img, b, gamma, beta):
    lla = np.float32(np.exp(log_log_alpha.astype(np.float32)))
    dtv = np.exp(log_dt.astype(np.float32)).astype(np.float32)
    z = (-lla.astype(np.complex64) + 1j * alpha_img.astype(np.complex64)) * dtv
    alpha = np.exp(z.astype(np.complex64))
    a_r = alpha.real.astype(np.float32)
    a_i = alpha.imag.astype(np.float32)
    ident = np.eye(P, dtype=np.float32)
    return a_r, a_i, ident


def kernel(x, W, log_log_alpha, log_dt, alpha_img, b, gamma, beta,
           u0_real, u0_imag, s0):
    from concourse.bass_utils import run_bass_kernel_spmd

    if "nc" not in _CACHE:
        _CACHE["nc"] = _build()
    nc = _CACHE["nc"]

    a_r, a_i, ident = _prep_host(W, log_log_alpha, log_dt, alpha_img, b, gamma, beta)

    in_maps = []
    for c in range(NCORES):
        hc, bh = c // NB, c % NB
        hs = slice(hc * P, (hc + 1) * P)
        bs = slice(bh * BLOC, (bh + 1) * BLOC)
        arh = a_r[hs]
        aih = a_i[hs]
        cvec = np.zeros((P, 8), np.float32)
        cvec[:, 0] = arh
        cvec[:, 1] = -arh
        cvec[:, 2] = aih * aih
        cvec[:, 3] = (b * gamma)[hs].astype(np.float32)
        cvec[:, 4] = (b * beta)[hs].astype(np.float32)
        st0 = np.zeros((P, 2, BLOC), np.float32)
        st0[:, 0] = (
            s0[bs].astype(np.float32) - u0_real[bs].astype(np.float32)
        ).T[hs, :]
        st0[:, 1] = (-u0_imag[bs].astype(np.float32)).T[hs, :] / aih[:, None]
        hsel = np.zeros((P, NH), np.float32)
        hsel[:, hc] = 1.0
        in_maps.append({
            "x": np.ascontiguousarray(x[bs].astype(np.float32)),
            "wt": np.ascontiguousarray(W[hs, :].T.astype(np.float32)),
            "ident": ident,
            "cvec": cvec,
            "st0": st0,
            "hsel": hsel,
        })

    res = run_bass_kernel_spmd(
        nc,
        in_maps,
        core_ids=list(range(NCORES)),
        trace=bool(int(os.environ.get("LIF_TRACE", "0"))),
    )
    _CACHE["last_res"] = res
    out = np.empty((B, T, H), np.float32)
    for c in range(NCORES):
        hc, bh = c // NB, c % NB
        o = res.results[c]["out"]  # [P, T, BLOC]
        out[bh * BLOC : (bh + 1) * BLOC, :, hc * P : (hc + 1) * P] = o.transpose(
            2, 1, 0
        )
    return out


# revision 16
# speedup vs baseline: 3.1299x; 1.9120x over previous
"""Trainium2 Bass kernel for nn_LIFcomplexLayer (v2: h-on-partition reshard).

Computes: Wx = x @ W.T ; BatchNorm(train stats over (B,T)) ; complex-decay
LIF recurrence with spike output.

Sharding: 8 cores = 4 h-chunks x 2 b-halves. Core c owns h in
[128*(c//2), 128*(c//2)+128) and batches [16*(c%2), 16*(c%2)+16).
With h on partitions, all per-neuron constants are per-partition [P,1]
scalars, so each recurrence step is 4 fused STT ops on one engine (DVE):
    cw   = Q*wt + d_t            (Q = a_i^2, wt = -u_i/a_i scaled imag state)
    wt'  = A*wt + negm           (A = a_r)
    v    = -A*negm + cw          (written into the drive column, in place)
    negm'= (v > 0.5) - v
Spike extraction ((v>0.5) in bulk) runs on GpSimd in T-chunks so the output
DMA overlaps the tail of the recurrence.

BN statistics are all-reduced across cores with a tiny [128, 8] collective
(each core fills its h-chunk slot; the pair of cores sharing an h-chunk sum).
"""

import sys

if "/opt/trn_rl_repo" not in sys.path:
    sys.path.insert(0, "/opt/trn_rl_repo")

import os
import numpy as np

B, T, I, H = 32, 2048, 512, 512
NCORES = 8
NH = 4                       # h-shards
NB = 2                       # b-shards
BLOC = B // NB               # 16 batches per core
P = 128                      # partitions = h per core
IC = I // P                  # 4 i-chunks
TC = 4                       # t-chunks per batch in phase A
TCH = T // TC                # 512 t per chunk
NTOT = float(B * T)          # BN sample count
BN_EPS = 1e-5

TSTEPS = int(os.environ.get("LIF_TSTEPS", str(T)))
OCH = 512                    # spike-output chunk (steps per out DMA)

_CACHE = {}


def _build():
    import concourse.bass as bass
    import concourse.bacc as bacc
    import concourse.tile as tile
    from concourse import mybir

    dt = mybir.dt
    f32 = dt.float32
    Alu = mybir.AluOpType
    Act = mybir.ActivationFunctionType

    from contextlib import ExitStack

    nc = bacc.Bacc(
        "TRN2", target_bir_lowering=False, debug=False, num_devices=NCORES
    )

    x_d = nc.dram_tensor("x", [BLOC, T, I], f32, kind="ExternalInput").ap()
    wt_d = nc.dram_tensor("wt", [I, P], f32, kind="ExternalInput").ap()
    ident_d = nc.dram_tensor("ident", [P, P], f32, kind="ExternalInput").ap()
    cvec_d = nc.dram_tensor("cvec", [P, 8], f32, kind="ExternalInput").ap()
    st0_d = nc.dram_tensor("st0", [P, 2, BLOC], f32, kind="ExternalInput").ap()
    hsel_d = nc.dram_tensor("hsel", [P, NH], f32, kind="ExternalInput").ap()
    out_d = nc.dram_tensor("out", [P, T, BLOC], f32, kind="ExternalOutput").ap()

    with tile.TileContext(nc) as tc, ExitStack() as ctx:
        consts = ctx.enter_context(tc.tile_pool(name="consts", bufs=1))
        big = ctx.enter_context(tc.tile_pool(name="big", bufs=1))
        xin = ctx.enter_context(tc.tile_pool(name="xin", bufs=2))
        xtp = ctx.enter_context(tc.tile_pool(name="xtp", bufs=2))
        tpool = ctx.enter_context(tc.tile_pool(name="psumT", bufs=4, space="PSUM"))
        mpool = ctx.enter_context(tc.tile_pool(name="psumM", bufs=2, space="PSUM"))
        trash_p = ctx.enter_context(tc.tile_pool(name="trash", bufs=2))
        small = ctx.enter_context(tc.tile_pool(name="small", bufs=1))
        state_p = ctx.enter_context(tc.tile_pool(name="state", bufs=1))
        scr = ctx.enter_context(tc.tile_pool(name="scr", bufs=2))
        dram = ctx.enter_context(tc.tile_pool(name="dram", bufs=1, space="DRAM"))

        wt_sb = consts.tile([P, IC, P], f32)        # [i(128p), ic, h]
        nc.sync.dma_start(wt_sb[:], wt_d.rearrange("(ic p) h -> p ic h", p=P))
        ident_sb = consts.tile([P, P], f32)
        nc.sync.dma_start(ident_sb[:], ident_d[:])
        cvec = consts.tile([P, 8], f32)             # A, nA, Q, bg, bb, ...
        nc.sync.dma_start(cvec[:], cvec_d[:])
        st0_sb = consts.tile([P, 2, BLOC], f32)     # negm0, wtil0
        nc.sync.dma_start(st0_sb[:], st0_d[:])
        hsel = consts.tile([P, NH], f32)            # one-hot column = my hc
        nc.sync.dma_start(hsel[:], hsel_d[:])

        A_ap = cvec[:, 0:1]
        nA_ap = cvec[:, 1:2]
        Q_ap = cvec[:, 2:3]
        bg_ap = cvec[:, 3:4]   # b * gamma
        bb_ap = cvec[:, 4:5]   # b * beta

        # Drive buffer: [h(128p), t, b] (t-major so per-step [P, BLOC] tiles
        # are contiguous; +1 dummy tail column to flush the scan pipeline).
        # Raw Wx -> BN'd drive -> v/spikes.
        wxbuf = big.tile([P, T + 1, BLOC], f32)
        sumS = small.tile([P, BLOC * TC], f32)
        sumQ = small.tile([P, BLOC * TC], f32)

        # ---- phase A: x load, PE transpose, matmul, raw sums ----
        for b in range(BLOC):
            for tcix in range(TC):
                xr = xin.tile([P, TC, I], f32)  # [t(128p), tt, i]
                nc.sync.dma_start(
                    xr[:],
                    x_d[b, tcix * TCH : (tcix + 1) * TCH, :].rearrange(
                        "(tt p) i -> p tt i", p=P
                    ),
                )
                xt = xtp.tile([P, IC, TCH], f32)  # [i(128p), ic, t]
                k = b * TC + tcix
                for ic in range(IC):
                    pt = tpool.tile([P, TCH], f32)
                    for tt in range(TC):
                        nc.tensor.transpose(
                            pt[:, tt * P : (tt + 1) * P],
                            xr[:, tt, ic * P : (ic + 1) * P],
                            ident_sb[:],
                        )
                    # gpsimd cannot read PSUM; rotate ACT/DVE only
                    if (k * IC + ic) % 2 == 0:
                        nc.scalar.copy(xt[:, ic, :], pt[:])
                    else:
                        nc.vector.tensor_copy(xt[:, ic, :], pt[:])
                pm = mpool.tile([P, TCH], f32)
                for ic in range(IC):
                    nc.tensor.matmul(
                        pm[:],
                        lhsT=wt_sb[:, ic, :],
                        rhs=xt[:, ic, :],
                        start=(ic == 0),
                        stop=(ic == IC - 1),
                    )
                dst = wxbuf[:, tcix * TCH : (tcix + 1) * TCH, b]
                nc.scalar.activation(
                    dst, pm[:], Act.Identity, accum_out=sumS[:, k : k + 1]
                )
                trash = trash_p.tile([P, TCH], f32)
                nc.vector.scalar_tensor_tensor(
                    trash[:],
                    dst,
                    1.0,
                    dst,
                    op0=Alu.bypass,
                    op1=Alu.mult,
                    accum_out=sumQ[:, k : k + 1],
                )

        # ---- phase B: stats all-reduce + BN finalize + apply ----
        stats = small.tile([P, 2, NH], f32)
        nc.vector.memset(stats[:], 0.0)
        ssum = small.tile([P, 2], f32)
        nc.vector.tensor_reduce(
            ssum[:, 0:1], sumS[:], axis=mybir.AxisListType.X, op=Alu.add
        )
        nc.vector.tensor_reduce(
            ssum[:, 1:2], sumQ[:], axis=mybir.AxisListType.X, op=Alu.add
        )
        # scatter into my h-chunk slot: stats[:, s, hc] = ssum[:, s]
        nc.vector.tensor_scalar(
            stats[:, 0, :], hsel[:], ssum[:, 0:1], None, op0=Alu.mult
        )
        nc.vector.tensor_scalar(
            stats[:, 1, :], hsel[:], ssum[:, 1:2], None, op0=Alu.mult
        )
        cc_in = dram.tile([P, 2 * NH], f32)
        cc_out = dram.tile([P, 2 * NH], f32)
        nc.sync.dma_start(cc_in[:], stats[:].rearrange("p a h -> p (a h)"))
        nc.gpsimd.collective_compute(
            "AllReduce",
            Alu.add,
            replica_groups=[list(range(NCORES))],
            ins=[cc_in.opt()],
            outs=[cc_out.opt()],
        )
        gstats = small.tile([P, 2, NH], f32)
        nc.sync.dma_start(gstats[:], cc_out[:].rearrange("p (a h) -> p a h", a=2))
        gsum = small.tile([P, 2], f32)
        # gather my slot back: multiply by one-hot and reduce
        tmp2 = small.tile([P, NH], f32)
        nc.vector.tensor_tensor(tmp2[:], gstats[:, 0, :], hsel[:], op=Alu.mult)
        nc.vector.tensor_reduce(
            gsum[:, 0:1], tmp2[:], axis=mybir.AxisListType.X, op=Alu.add
        )
        nc.vector.tensor_tensor(tmp2[:], gstats[:, 1, :], hsel[:], op=Alu.mult)
        nc.vector.tensor_reduce(
            gsum[:, 1:2], tmp2[:], axis=mybir.AxisListType.X, op=Alu.add
        )

        mean = small.tile([P, 1], f32)
        ex2 = small.tile([P, 1], f32)
        var = small.tile([P, 1], f32)
        inv = small.tile([P, 1], f32)
        gsc = small.tile([P, 1], f32)
        hof = small.tile([P, 1], f32)
        tmp = small.tile([P, 1], f32)
        nc.vector.tensor_scalar(mean[:], gsum[:, 0:1], 1.0 / NTOT, None, op0=Alu.mult)
        nc.vector.tensor_scalar(ex2[:], gsum[:, 1:2], 1.0 / NTOT, None, op0=Alu.mult)
        nc.vector.tensor_tensor(tmp[:], mean[:], mean[:], op=Alu.mult)
        nc.vector.tensor_tensor(var[:], ex2[:], tmp[:], op=Alu.subtract)
        nc.vector.tensor_scalar(var[:], var[:], BN_EPS, None, op0=Alu.add)
        nc.scalar.sqrt(tmp[:], var[:])
        nc.vector.reciprocal(inv[:], tmp[:])
        nc.vector.tensor_tensor(gsc[:], bg_ap, inv[:], op=Alu.mult)
        nc.vector.tensor_tensor(tmp[:], mean[:], gsc[:], op=Alu.mult)
        nc.vector.tensor_tensor(hof[:], bb_ap, tmp[:], op=Alu.subtract)

        # BN apply on big contiguous [P, TCH*BLOC] slices, DVE/ACT alternating
        for tcix in range(TC):
            sl = wxbuf[:, tcix * TCH : (tcix + 1) * TCH, :]
            if tcix % 2 == 0:
                nc.vector.tensor_scalar(
                    sl, sl, gsc[:], hof[:], op0=Alu.mult, op1=Alu.add
                )
            else:
                nc.scalar.activation(
                    sl, sl, Act.Identity, bias=hof[:], scale=gsc[:]
                )

        # ---- phase C: fused LIF scan (one DVE instruction per chain) ----
        # Step 0 uses the explicit (negm, wtil) form to produce v(0) and
        # wtil(0); the custom scan op then runs steps 1..T-1 (+1 dummy) per
        # chain at 9 cycles/step, writing spikes in place over the drive.
        from lif_ops import register_scan_op

        SCAN = register_scan_op(emit="s")

        negm = state_p.tile([P, BLOC], f32)
        wtil = state_p.tile([P, BLOC], f32)
        nc.vector.tensor_copy(negm[:], st0_sb[:, 0, :])
        nc.vector.tensor_copy(wtil[:], st0_sb[:, 1, :])

        # step 0 (explicit negm/wtil)
        col0 = wxbuf[:, 0, :]
        cw = scr.tile([P, BLOC], f32, tag="cw")
        nc.vector.scalar_tensor_tensor(
            cw[:], wtil[:], Q_ap, col0, op0=Alu.mult, op1=Alu.add
        )
        nc.vector.scalar_tensor_tensor(
            wtil[:], wtil[:], A_ap, negm[:], op0=Alu.mult, op1=Alu.add
        )
        nc.vector.scalar_tensor_tensor(
            col0, negm[:], nA_ap, cw[:], op0=Alu.mult, op1=Alu.add
        )

        # Seeds for the scan: N(0) = (v0>0.5) - v0, R(0) = Q * wtil(0),
        # laid out [P, BLOC, 2] so per-chain [P, 2] slices are contiguous.
        st_scan = state_p.tile([P, BLOC, 2], f32)
        nc.vector.scalar_tensor_tensor(
            st_scan[:, :, 0], col0, 0.5, col0, op0=Alu.is_gt, op1=Alu.subtract
        )
        nc.vector.tensor_scalar(
            st_scan[:, :, 1], wtil[:], Q_ap, None, op0=Alu.mult
        )
        # spikes for step 0 (in place over v(0); seeds already captured)
        nc.vector.tensor_scalar(col0, col0, 0.5, None, op0=Alu.is_gt)

        for c in range(BLOC):
            nc.vector._custom_dve(
                SCAN,
                out=wxbuf[:, 1 : TSTEPS + 1, c],
                in0=wxbuf[:, 1 : TSTEPS + 1, c],
                in1=st_scan[:, c, :],
                s0=A_ap,
                s1=Q_ap,
                imm2=0.5,
            )
        nc.sync.dma_start(out_d[:, 0:TSTEPS, :], wxbuf[:, 0:TSTEPS, :])

    nc.compile()
    return nc


def _prep_host(W, log_log_alpha, log_dt, alpha_img, b, gamma, beta):
    lla = np.float32(np.exp(log_log_alpha.astype(np.float32)))
    dtv = np.exp(log_dt.astype(np.float32)).astype(np.float32)
    z = (-lla.astype(np.complex64) + 1j * alpha_img.astype(np.complex64)) * dtv
    alpha = np.exp(z.astype(np.complex64))
    a_r = alpha.real.astype(np.float32)
    a_i = alpha.imag.astype(np.float32)
    ident = np.eye(P, dtype=np.float32)
    return a_r, a_i, ident


def kernel(x, W, log_log_alpha, log_dt, alpha_img, b, gamma, beta,
           u0_real, u0_imag, s0):
    from concourse.bass_utils import run_bass_kernel_spmd

    if "nc" not in _CACHE:
        _CACHE["nc"] = _build()
    nc = _CACHE["nc"]

    a_r, a_i, ident = _prep_host(W, log_log_alpha, log_dt, alpha_img, b, gamma, beta)

    in_maps = []
    for c in range(NCORES):
        hc, bh = c // NB, c % NB
        hs = slice(hc * P, (hc + 1) * P)
        bs = slice(bh * BLOC, (bh + 1) * BLOC)
        arh = a_r[hs]
        aih = a_i[hs]
        cvec = np.zeros((P, 8), np.float32)
        cvec[:, 0] = arh
        cvec[:, 1] = -arh
        cvec[:, 2] = aih * aih
        cvec[:, 3] = (b * gamma)[hs].astype(np.float32)
        cvec[:, 4] = (b * beta)[hs].astype(np.float32)
        st0 = np.zeros((P, 2, BLOC), np.float32)
        st0[:, 0] = (
            s0[bs].astype(np.float32) - u0_real[bs].astype(np.float32)
        ).T[hs, :]
        st0[:, 1] = (-u0_imag[bs].astype(np.float32)).T[hs, :] / aih[:, None]
        hsel = np.zeros((P, NH), np.float32)
        hsel[:, hc] = 1.0
        in_maps.append({
            "x": np.ascontiguousarray(x[bs].astype(np.float32)),
            "wt": np.ascontiguousarray(W[hs, :].T.astype(np.float32)),
            "ident": ident,
            "cvec": cvec,
            "st0": st0,
            "hsel": hsel,
        })

    res = run_bass_kernel_spmd(
        nc,
        in_maps,
        core_ids=list(range(NCORES)),
        trace=bool(int(os.environ.get("LIF_TRACE", "0"))),
    )
    _CACHE["last_res"] = res
    out = np.empty((B, T, H), np.float32)
    for c in range(NCORES):
        hc, bh = c // NB, c % NB
        o = res.results[c]["out"]  # [P, T, BLOC]
        out[bh * BLOC : (bh + 1) * BLOC, :, hc * P : (hc + 1) * P] = o.transpose(
            2, 1, 0
        )
    return out


# revision 22
# speedup vs baseline: 3.7909x; 1.2112x over previous
"""Trainium2 Bass kernel for nn_LIFcomplexLayer (v2: h-on-partition reshard).

Computes: Wx = x @ W.T ; BatchNorm(train stats over (B,T)) ; complex-decay
LIF recurrence with spike output.

Sharding: 8 cores = 4 h-chunks x 2 b-halves. Core c owns h in
[128*(c//2), 128*(c//2)+128) and batches [16*(c%2), 16*(c%2)+16).
With h on partitions, all per-neuron constants are per-partition [P,1]
scalars, so each recurrence step is 4 fused STT ops on one engine (DVE):
    cw   = Q*wt + d_t            (Q = a_i^2, wt = -u_i/a_i scaled imag state)
    wt'  = A*wt + negm           (A = a_r)
    v    = -A*negm + cw          (written into the drive column, in place)
    negm'= (v > 0.5) - v
Spike extraction ((v>0.5) in bulk) runs on GpSimd in T-chunks so the output
DMA overlaps the tail of the recurrence.

BN statistics are all-reduced across cores with a tiny [128, 8] collective
(each core fills its h-chunk slot; the pair of cores sharing an h-chunk sum).
"""

import sys

if "/opt/trn_rl_repo" not in sys.path:
    sys.path.insert(0, "/opt/trn_rl_repo")

import os
import numpy as np

B, T, I, H = 32, 2048, 512, 512
NCORES = 8
NH = 4                       # h-shards
NB = 2                       # b-shards
BLOC = B // NB               # 16 batches per core
P = 128                      # partitions = h per core
IC = I // P                  # 4 i-chunks
TC = 4                       # t-chunks per batch in phase A
TCH = T // TC                # 512 t per chunk
NTOT = float(B * T)          # BN sample count
BN_EPS = 1e-5

TSTEPS = int(os.environ.get("LIF_TSTEPS", str(T)))
OCH = 512                    # spike-output chunk (steps per out DMA)

_CACHE = {}


def _build():
    import concourse.bass as bass
    import concourse.bacc as bacc
    import concourse.tile as tile
    from concourse import mybir

    dt = mybir.dt
    f32 = dt.float32
    Alu = mybir.AluOpType
    Act = mybir.ActivationFunctionType

    from contextlib import ExitStack

    nc = bacc.Bacc(
        "TRN2", target_bir_lowering=False, debug=False, num_devices=NCORES
    )

    # x is pre-transposed on the host: [b, i, t]
    x_d = nc.dram_tensor("x", [BLOC, I, T], f32, kind="ExternalInput").ap()
    wt_d = nc.dram_tensor("wt", [I, P], f32, kind="ExternalInput").ap()
    cvec_d = nc.dram_tensor("cvec", [P, 8], f32, kind="ExternalInput").ap()
    st0_d = nc.dram_tensor("st0", [P, 2, BLOC], f32, kind="ExternalInput").ap()
    hsel_d = nc.dram_tensor("hsel", [P, NH], f32, kind="ExternalInput").ap()
    out_d = nc.dram_tensor("out", [P, T, BLOC], f32, kind="ExternalOutput").ap()

    with tile.TileContext(nc) as tc, ExitStack() as ctx:
        consts = ctx.enter_context(tc.tile_pool(name="consts", bufs=1))
        big = ctx.enter_context(tc.tile_pool(name="big", bufs=1))
        xtp = ctx.enter_context(tc.tile_pool(name="xtp", bufs=3))
        mpool = ctx.enter_context(tc.tile_pool(name="psumM", bufs=2, space="PSUM"))
        trash_p = ctx.enter_context(tc.tile_pool(name="trash", bufs=2))
        small = ctx.enter_context(tc.tile_pool(name="small", bufs=1))
        state_p = ctx.enter_context(tc.tile_pool(name="state", bufs=1))
        scr = ctx.enter_context(tc.tile_pool(name="scr", bufs=2))
        dram = ctx.enter_context(tc.tile_pool(name="dram", bufs=1, space="DRAM"))

        wt_sb = consts.tile([P, IC, P], f32)        # [i(128p), ic, h]
        nc.sync.dma_start(wt_sb[:], wt_d.rearrange("(ic p) h -> p ic h", p=P))
        cvec = consts.tile([P, 8], f32)             # A, nA, Q, bg, bb, ...
        nc.sync.dma_start(cvec[:], cvec_d[:])
        st0_sb = consts.tile([P, 2, BLOC], f32)     # negm0, wtil0
        nc.sync.dma_start(st0_sb[:], st0_d[:])
        hsel = consts.tile([P, NH], f32)            # one-hot column = my hc
        nc.sync.dma_start(hsel[:], hsel_d[:])

        A_ap = cvec[:, 0:1]
        nA_ap = cvec[:, 1:2]
        Q_ap = cvec[:, 2:3]
        bg_ap = cvec[:, 3:4]   # b * gamma
        bb_ap = cvec[:, 4:5]   # b * beta

        # Drive buffer: [h(128p), t, b] (t-major so per-step [P, BLOC] tiles
        # are contiguous; +1 dummy tail column to flush the scan pipeline).
        # Raw Wx -> BN'd drive -> v/spikes.
        wxbuf = big.tile([P, T + 1, BLOC], f32)
        sumS = small.tile([P, BLOC * TC], f32)
        sumQ = small.tile([P, BLOC * TC], f32)

        # ---- phase A: x^T load (host pre-transposed), matmul, raw sums ----
        for b in range(BLOC):
            for tcix in range(TC):
                xt = xtp.tile([P, IC, TCH], f32)  # [i(128p), ic, t]
                nc.sync.dma_start(
                    xt[:],
                    x_d[b, :, tcix * TCH : (tcix + 1) * TCH].rearrange(
                        "(ic p) t -> p ic t", p=P
                    ),
                )
                k = b * TC + tcix
                pm = mpool.tile([P, TCH], f32)
                for ic in range(IC):
                    nc.tensor.matmul(
                        pm[:],
                        lhsT=wt_sb[:, ic, :],
                        rhs=xt[:, ic, :],
                        start=(ic == 0),
                        stop=(ic == IC - 1),
                    )
                dst = wxbuf[:, tcix * TCH : (tcix + 1) * TCH, b]
                nc.scalar.activation(
                    dst, pm[:], Act.Identity, accum_out=sumS[:, k : k + 1]
                )
                trash = trash_p.tile([P, TCH], f32)
                nc.vector.scalar_tensor_tensor(
                    trash[:],
                    dst,
                    1.0,
                    dst,
                    op0=Alu.bypass,
                    op1=Alu.mult,
                    accum_out=sumQ[:, k : k + 1],
                )

        # ---- phase B: stats all-reduce + BN finalize + apply ----
        stats = small.tile([P, 2, NH], f32)
        nc.vector.memset(stats[:], 0.0)
        ssum = small.tile([P, 2], f32)
        nc.vector.tensor_reduce(
            ssum[:, 0:1], sumS[:], axis=mybir.AxisListType.X, op=Alu.add
        )
        nc.vector.tensor_reduce(
            ssum[:, 1:2], sumQ[:], axis=mybir.AxisListType.X, op=Alu.add
        )
        # scatter into my h-chunk slot: stats[:, s, hc] = ssum[:, s]
        nc.vector.tensor_scalar(
            stats[:, 0, :], hsel[:], ssum[:, 0:1], None, op0=Alu.mult
        )
        nc.vector.tensor_scalar(
            stats[:, 1, :], hsel[:], ssum[:, 1:2], None, op0=Alu.mult
        )
        cc_in = dram.tile([P, 2 * NH], f32)
        cc_out = dram.tile([P, 2 * NH], f32)
        nc.sync.dma_start(cc_in[:], stats[:].rearrange("p a h -> p (a h)"))
        nc.gpsimd.collective_compute(
            "AllReduce",
            Alu.add,
            replica_groups=[list(range(NCORES))],
            ins=[cc_in.opt()],
            outs=[cc_out.opt()],
        )
        gstats = small.tile([P, 2, NH], f32)
        nc.sync.dma_start(gstats[:], cc_out[:].rearrange("p (a h) -> p a h", a=2))
        gsum = small.tile([P, 2], f32)
        # gather my slot back: multiply by one-hot and reduce
        tmp2 = small.tile([P, NH], f32)
        nc.vector.tensor_tensor(tmp2[:], gstats[:, 0, :], hsel[:], op=Alu.mult)
        nc.vector.tensor_reduce(
            gsum[:, 0:1], tmp2[:], axis=mybir.AxisListType.X, op=Alu.add
        )
        nc.vector.tensor_tensor(tmp2[:], gstats[:, 1, :], hsel[:], op=Alu.mult)
        nc.vector.tensor_reduce(
            gsum[:, 1:2], tmp2[:], axis=mybir.AxisListType.X, op=Alu.add
        )

        mean = small.tile([P, 1], f32)
        ex2 = small.tile([P, 1], f32)
        var = small.tile([P, 1], f32)
        inv = small.tile([P, 1], f32)
        gsc = small.tile([P, 1], f32)
        hof = small.tile([P, 1], f32)
        tmp = small.tile([P, 1], f32)
        nc.vector.tensor_scalar(mean[:], gsum[:, 0:1], 1.0 / NTOT, None, op0=Alu.mult)
        nc.vector.tensor_scalar(ex2[:], gsum[:, 1:2], 1.0 / NTOT, None, op0=Alu.mult)
        nc.vector.tensor_tensor(tmp[:], mean[:], mean[:], op=Alu.mult)
        nc.vector.tensor_tensor(var[:], ex2[:], tmp[:], op=Alu.subtract)
        nc.vector.tensor_scalar(var[:], var[:], BN_EPS, None, op0=Alu.add)
        nc.scalar.sqrt(tmp[:], var[:])
        nc.vector.reciprocal(inv[:], tmp[:])
        nc.vector.tensor_tensor(gsc[:], bg_ap, inv[:], op=Alu.mult)
        nc.vector.tensor_tensor(tmp[:], mean[:], gsc[:], op=Alu.mult)
        nc.vector.tensor_tensor(hof[:], bb_ap, tmp[:], op=Alu.subtract)

        # BN apply on big contiguous [P, TCH*BLOC] slices, DVE/ACT alternating
        for tcix in range(TC):
            sl = wxbuf[:, tcix * TCH : (tcix + 1) * TCH, :]
            if tcix % 2 == 0:
                nc.vector.tensor_scalar(
                    sl, sl, gsc[:], hof[:], op0=Alu.mult, op1=Alu.add
                )
            else:
                nc.scalar.activation(
                    sl, sl, Act.Identity, bias=hof[:], scale=gsc[:]
                )

        # ---- phase C: fused LIF scan (one DVE instruction per chain) ----
        # Step 0 uses the explicit (negm, wtil) form to produce v(0) and
        # wtil(0); the custom scan op then runs steps 1..T-1 (+1 dummy) per
        # chain at 9 cycles/step, writing spikes in place over the drive.
        from lif_ops import register_scan_op

        SCAN = register_scan_op(emit="s")

        negm = state_p.tile([P, BLOC], f32)
        wtil = state_p.tile([P, BLOC], f32)
        nc.vector.tensor_copy(negm[:], st0_sb[:, 0, :])
        nc.vector.tensor_copy(wtil[:], st0_sb[:, 1, :])

        # step 0 (explicit negm/wtil)
        col0 = wxbuf[:, 0, :]
        cw = scr.tile([P, BLOC], f32, tag="cw")
        nc.vector.scalar_tensor_tensor(
            cw[:], wtil[:], Q_ap, col0, op0=Alu.mult, op1=Alu.add
        )
        nc.vector.scalar_tensor_tensor(
            wtil[:], wtil[:], A_ap, negm[:], op0=Alu.mult, op1=Alu.add
        )
        nc.vector.scalar_tensor_tensor(
            col0, negm[:], nA_ap, cw[:], op0=Alu.mult, op1=Alu.add
        )

        # Seeds for the scan: N(0) = (v0>0.5) - v0, R(0) = Q * wtil(0),
        # laid out [P, BLOC, 2] so per-chain [P, 2] slices are contiguous.
        st_scan = state_p.tile([P, BLOC, 2], f32)
        nc.vector.scalar_tensor_tensor(
            st_scan[:, :, 0], col0, 0.5, col0, op0=Alu.is_gt, op1=Alu.subtract
        )
        nc.vector.tensor_scalar(
            st_scan[:, :, 1], wtil[:], Q_ap, None, op0=Alu.mult
        )
        # spikes for step 0 (in place over v(0); seeds already captured)
        nc.vector.tensor_scalar(col0, col0, 0.5, None, op0=Alu.is_gt)

        for c in range(BLOC):
            nc.vector._custom_dve(
                SCAN,
                out=wxbuf[:, 1 : TSTEPS + 1, c],
                in0=wxbuf[:, 1 : TSTEPS + 1, c],
                in1=st_scan[:, c, :],
                s0=A_ap,
                s1=Q_ap,
                imm2=0.5,
            )
        nc.sync.dma_start(out_d[:, 0:TSTEPS, :], wxbuf[:, 0:TSTEPS, :])

    nc.compile()
    return nc


def _prep_host(W, log_log_alpha, log_dt, alpha_img, b, gamma, beta):
    lla = np.float32(np.exp(log_log_alpha.astype(np.float32)))
    dtv = np.exp(log_dt.astype(np.float32)).astype(np.float32)
    z = (-lla.astype(np.complex64) + 1j * alpha_img.astype(np.complex64)) * dtv
    alpha = np.exp(z.astype(np.complex64))
    a_r = alpha.real.astype(np.float32)
    a_i = alpha.imag.astype(np.float32)
    ident = np.eye(P, dtype=np.float32)
    return a_r, a_i, ident


def kernel(x, W, log_log_alpha, log_dt, alpha_img, b, gamma, beta,
           u0_real, u0_imag, s0):
    from concourse.bass_utils import run_bass_kernel_spmd

    if "nc" not in _CACHE:
        _CACHE["nc"] = _build()
    nc = _CACHE["nc"]

    a_r, a_i, ident = _prep_host(W, log_log_alpha, log_dt, alpha_img, b, gamma, beta)

    # host-side transpose of x per b-half: [16, I, T], shared by 4 h-shards
    xt_half = [
        np.ascontiguousarray(
            x[bh * BLOC : (bh + 1) * BLOC].astype(np.float32).transpose(0, 2, 1)
        )
        for bh in range(NB)
    ]

    in_maps = []
    for c in range(NCORES):
        hc, bh = c // NB, c % NB
        hs = slice(hc * P, (hc + 1) * P)
        bs = slice(bh * BLOC, (bh + 1) * BLOC)
        arh = a_r[hs]
        aih = a_i[hs]
        cvec = np.zeros((P, 8), np.float32)
        cvec[:, 0] = arh
        cvec[:, 1] = -arh
        cvec[:, 2] = aih * aih
        cvec[:, 3] = (b * gamma)[hs].astype(np.float32)
        cvec[:, 4] = (b * beta)[hs].astype(np.float32)
        st0 = np.zeros((P, 2, BLOC), np.float32)
        st0[:, 0] = (
            s0[bs].astype(np.float32) - u0_real[bs].astype(np.float32)
        ).T[hs, :]
        st0[:, 1] = (-u0_imag[bs].astype(np.float32)).T[hs, :] / aih[:, None]
        hsel = np.zeros((P, NH), np.float32)
        hsel[:, hc] = 1.0
        in_maps.append({
            "x": xt_half[bh],
            "wt": np.ascontiguousarray(W[hs, :].T.astype(np.float32)),
            "cvec": cvec,
            "st0": st0,
            "hsel": hsel,
        })

    res = run_bass_kernel_spmd(
        nc,
        in_maps,
        core_ids=list(range(NCORES)),
        trace=bool(int(os.environ.get("LIF_TRACE", "0"))),
    )
    _CACHE["last_res"] = res
    out = np.empty((B, T, H), np.float32)
    for c in range(NCORES):
        hc, bh = c // NB, c % NB
        o = res.results[c]["out"]  # [P, T, BLOC]
        out[bh * BLOC : (bh + 1) * BLOC, :, hc * P : (hc + 1) * P] = o.transpose(
            2, 1, 0
        )
    return out
